# revision 15
# baseline (speedup 1.0000x reference)
"""Trainium2 Bass kernel for nn_ContinuousOutputGenerator.

Math (per batch element b):
    proj = gelu(states @ W1 + b1) @ W2 + b2                      [N, O]
    w_u[n, g=(i,j)] = exp(-((gx_i-px_n)^2 + (gy_j-py_n)^2)/bw)   [N, G]
    out[g, :] = sum_n w_u[n, g] * proj[n, :] / (sum_n w_u[n, g] + eps)

Key structure:
  * RBF kernel matrix is SEPARABLE over the 64x64 grid:
        w_u[n, (i,j)] = A[n,i] * B[n,j]
    A/B are exp(-(g-p)^2/bw) tables [N, 64]; w_u tiles are built with a
    broadcast outer-product multiply on DVE/GPSIMD.
  * Normalizer S = A^T @ B accumulated as one small matmul; normalization is
    a per-row scale of the pooled output by 1/(S+eps).
  * All matmul operands are bf16: the fp32r baseline was LDWEIGHTS-bound
    (fp32 stationary loads get no FastWeightLoad => ~204ns x 1344 loads
    covered the whole 275us span). bf16 halves the weight-load stream and
    lets it hide under the matmul stream; fp32 accumulation in PSUM keeps
    the normalizer exact enough (worst-case rel err ~1e-2 < 2e-2 gate).
  * states are transposed + cast to bf16 on host => no PE transposes, half
    the DMA.
  * A/B exps: DVE/GPSIMD compute (g-p)^2, ACT does Exp only => no
    Square<->Exp<->Gelu activation-table thrash.

Sharding: data-parallel over batch. 8 batch elements -> 8 NeuronCores, MLP
weights replicated. Each core runs the identical program on its own slice.
"""

import sys
from contextlib import ExitStack

import numpy as np

if "/opt/trn_rl_repo" not in sys.path:
    sys.path.insert(0, "/opt/trn_rl_repo")

import concourse.bass as bass  # noqa: E402
import concourse.tile as tile  # noqa: E402
from concourse import bacc, bass_utils, mybir  # noqa: E402

F32 = mybir.dt.float32
BF16 = mybir.dt.bfloat16
AF = mybir.ActivationFunctionType

# Problem shape (hardcoded per contract)
B, N, D, H, O = 8, 4096, 256, 512, 256
GRID = 64
G = GRID * GRID
NT = N // 128          # 32 n-tiles of 128 entities
NCHUNK = 8             # MLP processes n in chunks of 512
CSUB = 4               # 128-row subtiles per chunk
GCHUNK = 4             # pooling g-chunks of 1024 grid points
GG = G // GCHUNK       # 1024
IPC = GRID // GCHUNK   # 16 i-values per g-chunk
BW = 0.1
EPS = 1e-8


def _body(tc, aps, out_ap):
    nc = tc.nc
    with ExitStack() as ctx:
        # ---------------- persistent SBUF ----------------
        const = ctx.enter_context(tc.tile_pool(name="const", bufs=1))
        w1k = [const.tile([128, H], BF16, tag=f"w1k{k}", name=f"w1k{k}") for k in range(2)]
        w2k = [const.tile([128, O], BF16, tag=f"w2k{k}", name=f"w2k{k}") for k in range(4)]
        q2x_sb = const.tile([128, NT * GRID], F32, tag="q2x")
        q2y_sb = const.tile([128, NT * GRID], F32, tag="q2y")
        b2b_sb = const.tile([128, O], F32, tag="b2b")
        b1_sb = const.tile([128, 4], F32, tag="b1")
        s_sb = const.tile([GRID, GRID], F32, tag="s_sb")
        r_sb = const.tile([GRID, GRID], F32, tag="r_sb")
        r_t = const.tile([128, NT], F32, tag="r_t")

        ab = ctx.enter_context(tc.tile_pool(name="ab", bufs=1))
        a_all = ab.tile([128, NT * GRID], BF16, tag="a_all")
        b_all = ab.tile([128, NT * GRID], BF16, tag="b_all")

        projp = ctx.enter_context(tc.tile_pool(name="projp", bufs=1))
        proj = projp.tile([128, NT * O], BF16, tag="proj")

        dram = ctx.enter_context(tc.tile_pool(name="dram", bufs=1, space="DRAM"))
        scr = dram.tile([G], F32, tag="scr")

        # ---------------- const DMAs ----------------
        for k in range(2):
            nc.sync.dma_start(w1k[k][:], aps["W1"][k * 128 : (k + 1) * 128, :])
        for k in range(4):
            nc.sync.dma_start(w2k[k][:], aps["W2"][k * 128 : (k + 1) * 128, :])
        nc.sync.dma_start(q2x_sb[:], aps["q2x"][:])
        nc.sync.dma_start(q2y_sb[:], aps["q2y"][:])
        nc.sync.dma_start(b2b_sb[:], aps["b2b"][:])
        nc.sync.dma_start(b1_sb[:], aps["b1"].rearrange("(m p) -> p m", p=128))

        # RBF factor tables, one big Exp per axis (no table thrash):
        #   A[n,i] = exp(-10*(g_i - px_n)^2), B likewise for y.
        nc.scalar.activation(a_all[:], q2x_sb[:], AF.Exp, scale=-1.0 / BW)
        nc.scalar.activation(b_all[:], q2y_sb[:], AF.Exp, scale=-1.0 / BW)

        # ---------------- A/B tables + MLP (chunked, overlapped) ----------------
        stp = ctx.enter_context(tc.tile_pool(name="stp", bufs=4))
        hT = ctx.enter_context(tc.tile_pool(name="hT", bufs=2))
        tmp = ctx.enter_context(tc.tile_pool(name="tmp", bufs=8))

        with (
            tc.tile_pool(name="ps_h", bufs=2, space="PSUM") as ps_h,
            tc.tile_pool(name="ps_p", bufs=2, space="PSUM") as ps_p,
            tc.tile_pool(name="ps_s", bufs=1, space="PSUM") as ps_s,
        ):
            for c in range(NCHUNK):
                # states chunk in (already [d, n] bf16 from host)
                sT = [stp.tile([128, 512], BF16, tag=f"sT{k}", name=f"sT{k}") for k in range(2)]
                for k in range(2):
                    nc.sync.dma_start(
                        sT[k][:],
                        aps["statesT"][k * 128 : (k + 1) * 128, c * 512 : (c + 1) * 512],
                    )

                # MM1 + exact GELU: hT[m] = gelu(W1^T sT + b1), [h=512, n=512]
                hts = [hT.tile([128, 512], BF16, tag=f"hT{m}", name=f"hT{m}") for m in range(4)]
                for m in range(4):
                    ph = ps_h.tile([128, 512], F32, tag="ph")
                    for k in range(2):
                        nc.tensor.matmul(
                            ph[:],
                            w1k[k][:, m * 128 : (m + 1) * 128],
                            sT[k][:],
                            start=(k == 0),
                            stop=(k == 1),
                        )
                    nc.scalar.activation(
                        hts[m][:], ph[:], AF.Gelu, bias=b1_sb[:, m : m + 1]
                    )

                # MM2 + bias: proj[n_tile] = hT^T W2 + b2, [n=128, o=256]
                for s in range(CSUB):
                    a = c * CSUB + s
                    pp = ps_p.tile([128, O], F32, tag="pp")
                    for k in range(4):
                        nc.tensor.matmul(
                            pp[:],
                            hts[k][:, s * 128 : (s + 1) * 128],
                            w2k[k][:],
                            start=(k == 0),
                            stop=(k == 3),
                        )
                    nc.vector.tensor_add(
                        proj[:, a * O : (a + 1) * O], pp[:], b2b_sb[:]
                    )

            # normalizer S = A^T @ B (bf16 in, fp32 acc), R = 1/(S+eps)
            ps = ps_s.tile([GRID, GRID], F32, tag="ps_s")
            for a in range(NT):
                nc.tensor.matmul(
                    ps[:],
                    a_all[:, a * GRID : (a + 1) * GRID],
                    b_all[:, a * GRID : (a + 1) * GRID],
                    start=(a == 0),
                    stop=(a == NT - 1),
                )
            nc.vector.tensor_scalar_add(s_sb[:], ps[:], EPS)
            nc.vector.reciprocal(r_sb[:], s_sb[:])
            # repartition R [64i, 64j] -> [128 part, 32 g-tiles] via DRAM
            nc.sync.dma_start(scr[:].rearrange("(i j) -> i j", i=GRID), r_sb[:])
            nc.sync.dma_start(r_t[:], scr[:].rearrange("(t p) -> p t", p=128))

        # ---------------- pooling: out = (w_u^T proj) * R ----------------
        wup = ctx.enter_context(tc.tile_pool(name="wup", bufs=10))
        osbp = ctx.enter_context(tc.tile_pool(name="osbp", bufs=4))
        with tc.tile_pool(name="ps_acc", bufs=2, space="PSUM") as ps_acc:
            for gc in range(GCHUNK):
                accs = [ps_acc.tile([128, 512], F32, tag=f"acc{t}", name=f"acc{t}") for t in range(4)]
                for a in range(NT):
                    wu = wup.tile([128, GG], BF16, tag="wu")
                    i0 = a * GRID + gc * IPC
                    a3 = a_all[:, i0 : i0 + IPC][:, :, None].broadcast_to(
                        [128, IPC, GRID]
                    )
                    b3 = b_all[:, a * GRID : (a + 1) * GRID][:, None, :].broadcast_to(
                        [128, IPC, GRID]
                    )
                    wu3 = wu[:].rearrange("p (i j) -> p i j", i=IPC)
                    eng = nc.gpsimd if a % 5 in (1, 3) else nc.vector
                    eng.tensor_mul(wu3, a3, b3)
                    for m in range(8):
                        # start=True clears the whole PSUM bank, so only the
                        # first matmul into each bank may set it; the second
                        # half lands on cleared has_written bits and overwrites.
                        nc.tensor.matmul(
                            accs[m // 2][:, (m % 2) * O : (m % 2 + 1) * O],
                            wu[:, m * 128 : (m + 1) * 128],
                            proj[:, a * O : (a + 1) * O],
                            start=(a == 0 and m % 2 == 0),
                            stop=(a == NT - 1),
                        )
                for t in range(4):
                    osb = osbp.tile([128, 512], F32, tag="osb")
                    for half in range(2):
                        gt = gc * 8 + t * 2 + half
                        nc.vector.tensor_scalar_mul(
                            osb[:, half * O : (half + 1) * O],
                            accs[t][:, half * O : (half + 1) * O],
                            r_t[:, gt : gt + 1],
                        )
                    r0 = (gc * 4 + t) * 256
                    nc.sync.dma_start(
                        out_ap[r0 : r0 + 256, :].rearrange("(a p) o -> p a o", a=2),
                        osb[:].rearrange("p (a o) -> p a o", a=2),
                    )


def build_module():
    nc = bacc.Bacc("TRN2", target_bir_lowering=False, debug=False, num_devices=B)
    aps = {
        "statesT": nc.dram_tensor("statesT", (D, N), BF16, kind="ExternalInput").ap(),
        "W1": nc.dram_tensor("W1", (D, H), BF16, kind="ExternalInput").ap(),
        "b1": nc.dram_tensor("b1", (H,), F32, kind="ExternalInput").ap(),
        "W2": nc.dram_tensor("W2", (H, O), BF16, kind="ExternalInput").ap(),
        "b2b": nc.dram_tensor("b2b", (128, O), F32, kind="ExternalInput").ap(),
        "q2x": nc.dram_tensor("q2x", (128, NT * GRID), F32, kind="ExternalInput").ap(),
        "q2y": nc.dram_tensor("q2y", (128, NT * GRID), F32, kind="ExternalInput").ap(),
    }
    out_ap = nc.dram_tensor("out", (G, O), F32, kind="ExternalOutput").ap()
    with tile.TileContext(nc) as tc:
        _body(tc, aps, out_ap)
    nc.compile()
    return nc


_NC = None


def _get_nc():
    global _NC
    if _NC is None:
        _NC = build_module()
    return _NC


def make_in_maps(inputs):
    import ml_dtypes

    states = np.asarray(inputs["entity_states"], np.float32)
    pos = np.asarray(inputs["entity_positions"], np.float32)
    W1 = np.ascontiguousarray(np.asarray(inputs["W1"], np.float32)).astype(
        ml_dtypes.bfloat16
    )
    b1 = np.ascontiguousarray(np.asarray(inputs["b1"], np.float32))
    W2 = np.ascontiguousarray(np.asarray(inputs["W2"], np.float32)).astype(
        ml_dtypes.bfloat16
    )
    b2 = np.asarray(inputs["b2"], np.float32)

    # [B, N, D] -> per-core [D, N] bf16 (kills on-device transposes)
    statesT = np.ascontiguousarray(states.transpose(0, 2, 1)).astype(
        ml_dtypes.bfloat16
    )

    g = np.linspace(-1.0, 1.0, GRID).astype(np.float32)
    b2b = np.ascontiguousarray(np.tile(b2[None, :], (128, 1)))
    # q2{x,y}[p, a*64+i] = (g_i - p{x,y}[a*128+p])^2
    pr = pos.reshape(B, NT, 128, 2)
    q2 = (g[None, None, None, :, None] - pr[:, :, :, None, :]) ** 2  # [B,NT,128,64,2]
    q2 = q2.transpose(4, 0, 2, 1, 3).reshape(2, B, 128, NT * GRID)
    q2x = np.ascontiguousarray(q2[0])
    q2y = np.ascontiguousarray(q2[1])
    return [
        {
            "statesT": statesT[b],
            "W1": W1,
            "b1": b1,
            "W2": W2,
            "b2b": b2b,
            "q2x": q2x[b],
            "q2y": q2y[b],
        }
        for b in range(B)
    ]


def run(inputs, trace=False, **kw):
    nc = _get_nc()
    res = bass_utils.run_bass_kernel_spmd(
        nc, make_in_maps(inputs), core_ids=list(range(B)), trace=trace, **kw
    )
    out = np.stack([r["out"] for r in res.results], axis=0)
    return out, res


def kernel(**inputs) -> np.ndarray:
    out, _ = run(inputs, trace=False)
    return out
